# revision 3
# baseline (speedup 1.0000x reference)
"""Trainium2 Bass kernel for BinarizeConv2dSDP.

Reference math (forward only):
    w    = rsqrt(m^2 + sum_k z_k^2/100) * (m + rv @ z)   elementwise over N
    bw   = sign(w)          -- the positive rsqrt factor drops out of sign()
    ba   = sign(x)
    out  = conv2d(ba, bw, pad=1, NCHW/OIHW) * alpha[o]

So on device we compute bw = sign(M + sum_k rv[k] * Z[k]), ba = sign(x),
then a 3x3 pad-1 conv as 9 shifted fp8 matmuls accumulating in PSUM
(values are exactly +-1 -> fp8 e4m3 matmuls with f32 PSUM accumulation
are bit-exact), and scale by alpha during the PSUM->SBUF copy.

Sharding across the 8 cores:
  - conv: data-parallel over batch (8 images per core)
  - weight computation: sharded over out-channels (32 per core, so each
    core reads only its 1/8 slice of Z), binarized pieces AllGathered.

Layouts:
  - activations: per image, SBUF tile [128 part(c%128), 2 (c chunk), 912]
    fp8, a 30x30 zero-padded image per channel (912 = 900 padded to a
    multiple of 16 for the DoubleRow AP step constraint).
  - weights: SBUF [128 part(c%128), 2 (c chunk), 9 (tap), 256 (o)] fp8.
  - conv: out[o, q] += sum_c bw[c, t, o] * act[c, q + off(t)] with
    q on a 30-wide grid (junk columns x=28,29 never DMA'd out).
"""

import sys

for _p in ("/opt/trn_rl_repo",):
    if _p not in sys.path:
        sys.path.insert(0, _p)

import numpy as np

import concourse.bass as bass
import concourse.bacc as bacc
import concourse.tile as tile
from concourse import mybir
from concourse.bass_utils import run_bass_kernel_spmd

N_CORES = 8
B = 64          # full batch
B_SH = B // N_CORES
C = 256         # in channels
O = 256         # out channels
O_SH = O // N_CORES
K = 8           # SDP rank
KK = 9          # 3x3 taps
H = 28
HP = 30         # padded row width
PADW = 912      # 30*30 = 900 padded to %16 for DoubleRow AP step
F32 = mybir.dt.float32
FP8 = mybir.dt.float8e4

USE_FP8_DR = True  # fp8 DoubleRow matmuls; False -> bf16 fallback
ACT_DT = FP8 if USE_FP8_DR else mybir.dt.bfloat16


def _build_kernel(tc, x_t, m_t, z_t, a_t, rv_t, out_t):
    nc = tc.nc
    import contextlib

    ctx = contextlib.ExitStack()
    consts = ctx.enter_context(tc.tile_pool(name="consts", bufs=1))
    zpool = ctx.enter_context(tc.tile_pool(name="zpool", bufs=3))
    wtmp = ctx.enter_context(tc.tile_pool(name="wtmp", bufs=2))
    wpool = ctx.enter_context(tc.tile_pool(name="wpool", bufs=1))
    stage = ctx.enter_context(tc.tile_pool(name="stage", bufs=3))
    acts = ctx.enter_context(tc.tile_pool(name="acts", bufs=1))
    outp = ctx.enter_context(tc.tile_pool(name="outp", bufs=3))
    psums = ctx.enter_context(tc.tile_pool(name="psums", bufs=4, space="PSUM"))
    dram = ctx.enter_context(tc.tile_pool(name="dram", bufs=1, space="DRAM"))

    with ctx:
        # ---- constants ----
        rv_sb = consts.tile([128, K], F32, name="rv_sb")
        nc.sync.dma_start(rv_sb, rv_t.ap().to_broadcast((128, K)))

        alpha_sb = consts.tile([128, 2], F32, name="alpha_sb")
        nc.sync.dma_start(
            alpha_sb, a_t.ap().rearrange("(oc p) a b -> p (oc a b)", p=128)
        )

        # ---- weight phase: wsum = M + sum_k rv[k] * Z[k] on this core's
        # o-slice, in [c_low, cc, o, t] layout (c on partitions) ----
        m_sb = wpool.tile([128, 2, O_SH, KK], F32, name="m_sb")
        m_src = m_t.ap().rearrange("o (cc p) ky kx -> p cc o (ky kx)", p=128)
        for cc in range(2):
            nc.sync.dma_start(m_sb[:, cc], m_src[:, cc])

        acc = wpool.tile([128, 2, O_SH, KK], F32, name="acc")
        for k in range(K):
            z_k = zpool.tile([128, 2, O_SH, KK], F32, name="z_k", tag="z_k")
            z_src = z_t.ap()[k].rearrange(
                "o (cc p) ky kx -> p cc o (ky kx)", p=128
            )
            for cc in range(2):
                nc.sync.dma_start(z_k[:, cc], z_src[:, cc])
            if k == 0:
                # acc = z_0 * rv_0 (ACT engine: copy with per-partition scale)
                nc.scalar.mul(acc, z_k, mul=rv_sb[:, 0:1])
            else:
                zs = wtmp.tile([128, 2, O_SH, KK], F32, name="zs", tag="zs")
                nc.scalar.mul(zs, z_k, mul=rv_sb[:, k : k + 1])
                nc.vector.tensor_add(acc, acc, zs)
        nc.vector.tensor_add(acc, acc, m_sb)

        # sign -> fp8 piece, permuting (o, t) -> (t, o) inside the write AP
        piece = wpool.tile([128, 2, KK, O_SH], ACT_DT, name="piece")
        nc.scalar.sign(piece.rearrange("p cc t o -> p cc o t"), acc)

        # ---- AllGather pieces through DRAM ----
        piece_dram = dram.tile([128, 2 * KK * O_SH], ACT_DT, name="piece_dram")
        nc.sync.dma_start(piece_dram, piece)
        gath = dram.tile(
            [N_CORES, 128, 2 * KK * O_SH], ACT_DT, name="gath", addr_space="Shared"
        )
        nc.gpsimd.collective_compute(
            "AllGather",
            mybir.AluOpType.bypass,
            replica_groups=[list(range(N_CORES))],
            ins=[piece_dram.opt()],
            outs=[gath.opt()],
        )

        # ---- load full weights [c_low, cc, t, (core o)] ----
        wt = consts.tile([128, 2, KK, N_CORES, O_SH], ACT_DT, name="wt")
        gv = gath.rearrange("r p (cc t o) -> p cc t r o", cc=2, t=KK)
        for cc in range(2):
            for t in range(KK):
                nc.sync.dma_start(wt[:, cc, t], gv[:, cc, t])
        wt = wt.rearrange("p cc t r o -> p cc t (r o)")

        # ---- activations: sign(x) into zero-padded fp8 images ----
        act_tiles = []
        for n in range(B_SH):
            xst = stage.tile([128, 2, H * H], F32, name="xst", tag="xst")
            nc.sync.dma_start(
                xst, x_t.ap()[n].rearrange("(cc p) h w -> p cc (h w)", p=128)
            )
            a_n = acts.tile([128, 2, PADW], ACT_DT, name=f"a{n}", tag=f"a{n}")
            nc.vector.memset(a_n, 0.0)
            interior = a_n[:, :, 31 : 31 + 28 * HP].rearrange(
                "p cc (r xx) -> p cc r xx", xx=HP
            )[:, :, :, :28]
            nc.scalar.sign(interior, xst.rearrange("p cc (h w) -> p cc h w", w=28))
            act_tiles.append(a_n)

        # ---- conv: 9 shifted matmuls per (image, o-chunk, half) ----
        for n in range(B_SH):
            a_n = act_tiles[n]
            for oc in range(2):
                for half in range(2):
                    ps = psums.tile([128, 420], F32, name="ps", tag="ps")
                    for t in range(KK):
                        dy, dx = divmod(t, 3)
                        off = (half * 14 + dy) * HP + dx
                        lhsT = wt[:, :, t, oc * 128 : (oc + 1) * 128]
                        rhs = a_n[:, :, off : off + 420]
                        if USE_FP8_DR:
                            nc.tensor.matmul(
                                ps,
                                lhsT,
                                rhs,
                                start=(t == 0),
                                stop=(t == KK - 1),
                                perf_mode=mybir.MatmulPerfMode.DoubleRow,
                            )
                        else:
                            for cc in range(2):
                                nc.tensor.matmul(
                                    ps,
                                    lhsT[:, cc],
                                    rhs[:, cc],
                                    start=(t == 0 and cc == 0),
                                    stop=(t == KK - 1 and cc == 1),
                                )
                    ob = outp.tile([128, 392], F32, name="ob", tag="ob")
                    nc.scalar.activation(
                        ob.rearrange("p (r xx) -> p r xx", xx=28),
                        ps.rearrange("p (r xx) -> p r xx", xx=HP)[:, :, :28],
                        mybir.ActivationFunctionType.Copy,
                        scale=alpha_sb[:, oc : oc + 1],
                    )
                    dst = out_t.ap()[n].rearrange(
                        "(oc2 p) h w -> p oc2 (h w)", p=128
                    )[:, oc, half * 392 : (half + 1) * 392]
                    nc.sync.dma_start(dst, ob)


_PROGRAM = None


def build_program():
    global _PROGRAM
    if _PROGRAM is not None:
        return _PROGRAM
    nc = bacc.Bacc(
        "TRN2",
        target_bir_lowering=False,
        debug=False,
        enable_asserts=True,
        num_devices=N_CORES,
    )
    x_t = nc.dram_tensor("x", [B_SH, C, H, H], F32, kind="ExternalInput")
    m_t = nc.dram_tensor("M", [O_SH, C, 3, 3], F32, kind="ExternalInput")
    z_t = nc.dram_tensor("Z", [K, O_SH, C, 3, 3], F32, kind="ExternalInput")
    a_t = nc.dram_tensor("alpha", [O, 1, 1], F32, kind="ExternalInput")
    rv_t = nc.dram_tensor("rv", [1, K], F32, kind="ExternalInput")
    out_t = nc.dram_tensor("out", [B_SH, O, H, H], F32, kind="ExternalOutput")

    with tile.TileContext(nc) as tc:
        _build_kernel(tc, x_t, m_t, z_t, a_t, rv_t, out_t)
    nc.compile()
    _PROGRAM = nc
    return nc


def make_in_maps(x, M, Z, alpha, rv):
    x = np.ascontiguousarray(np.asarray(x, dtype=np.float32))
    M = np.ascontiguousarray(np.asarray(M, dtype=np.float32))
    Z = np.ascontiguousarray(np.asarray(Z, dtype=np.float32))
    alpha = np.ascontiguousarray(np.asarray(alpha, dtype=np.float32))
    rv = np.ascontiguousarray(np.asarray(rv, dtype=np.float32))
    in_maps = []
    for i in range(N_CORES):
        in_maps.append(
            {
                "x": np.ascontiguousarray(x[i * B_SH : (i + 1) * B_SH]),
                "M": np.ascontiguousarray(M[i * O_SH : (i + 1) * O_SH]),
                "Z": np.ascontiguousarray(Z[:, i * O_SH : (i + 1) * O_SH]),
                "alpha": alpha,
                "rv": rv,
            }
        )
    return in_maps


def kernel(x, M, Z, alpha, rv, trace=False):
    nc = build_program()
    in_maps = make_in_maps(x, M, Z, alpha, rv)
    res = run_bass_kernel_spmd(
        nc, in_maps, core_ids=list(range(N_CORES)), trace=trace
    )
    out = np.concatenate([res.results[i]["out"] for i in range(N_CORES)], axis=0)
    if trace:
        kernel.last_results = res
    return out


if __name__ == "__main__":
    # smoke-build
    build_program()
    print("program built ok")


# revision 6
# speedup vs baseline: 1.5940x; 1.5940x over previous
"""Trainium2 Bass kernel for BinarizeConv2dSDP.

Reference math (forward only):
    w    = rsqrt(m^2 + sum_k z_k^2/100) * (m + rv @ z)   elementwise
    bw   = sign(w)        -- the positive rsqrt factor drops out of sign()
    ba   = sign(x)
    out  = conv2d(ba, bw, pad=1, NCHW/OIHW) * alpha[o]

Device computation: bw = sign(M + sum_k rv[k]*Z[k]), ba = sign(x), then the
3x3 pad-1 conv as 9 shifted fp8 DoubleRow matmuls accumulating in PSUM
(everything is +-1, so fp8 e4m3 with f32 PSUM accumulation is bit-exact),
alpha folded into the PSUM->SBUF copy.

Sharding (8 cores, no collectives): 2D grid, batch 4-way x out-channel
2-way. Core i handles images [16*(i//2), 16*(i//2)+16) and out-channels
[128*(i%2), 128*(i%2)+128). Each core reads only its Z/M/alpha o-half and
its x batch-quarter; outputs are disjoint.

Per-core layouts:
  - z_k, m, wsum: [128 part(o), 2304 (c*9+t)] f32 -- natural Z order, so
    all weight DMAs are fully contiguous.
  - weight sum on ACT (muls by rv[k]) + DVE (sequential add chain, same
    f32 order as the reference dot).
  - sign -> w8 [128(o), 2304] fp8; 18 PE transposes (matmul with fp8
    identity rhs, lhsT = stride-9 column slice) -> W [128 part(c_low),
    9 tap, 2 c-chunk, 128 o] fp8.
  - activations: per image [128 part(c_low), 2 c-chunk, 912] fp8 zero-
    padded 30x30 images (912 = 900 rounded up to %16 for the DoubleRow
    AP-step constraint); conv output on a 30-wide grid, junk columns
    skipped at the output DMA.
"""

import sys

for _p in ("/opt/trn_rl_repo",):
    if _p not in sys.path:
        sys.path.insert(0, _p)

import contextlib

import numpy as np

import concourse.bass as bass
import concourse.bacc as bacc
import concourse.tile as tile
from concourse import mybir
from concourse.bass_utils import run_bass_kernel_spmd

N_CORES = 8
B = 64
B_SH = 16       # images per core (batch/4)
C = 256         # in channels
O = 256
O_SH = 128      # out channels per core (o/2)
K = 8           # SDP rank
KK = 9          # 3x3 taps
CT = C * KK     # 2304
H = 28
HP = 30         # padded row width
PADW = 912      # 30*30=900 padded to %16
F32 = mybir.dt.float32
FP8 = mybir.dt.float8e4


def _build_kernel(tc, x_t, m_t, z_t, a_t, rv_t, eye_t, out_t):
    nc = tc.nc
    ctx = contextlib.ExitStack()
    consts = ctx.enter_context(tc.tile_pool(name="consts", bufs=1))
    zpool = ctx.enter_context(tc.tile_pool(name="zpool", bufs=1))
    wpool = ctx.enter_context(tc.tile_pool(name="wpool", bufs=1))
    stage = ctx.enter_context(tc.tile_pool(name="stage", bufs=4))
    acts = ctx.enter_context(tc.tile_pool(name="acts", bufs=1))
    outp = ctx.enter_context(tc.tile_pool(name="outp", bufs=4))
    psums = ctx.enter_context(tc.tile_pool(name="psums", bufs=4, space="PSUM"))
    pst = ctx.enter_context(tc.tile_pool(name="pst", bufs=2, space="PSUM"))

    with ctx:
        # ---- tiny constants (gpsimd SWDGE queue; keep HW rings free) ----
        rv_sb = consts.tile([128, K], F32, name="rv_sb")
        nc.gpsimd.dma_start(rv_sb, rv_t.ap().to_broadcast((128, K)))
        alpha_sb = consts.tile([128, 1], F32, name="alpha_sb")
        nc.gpsimd.dma_start(alpha_sb, a_t.ap().rearrange("p a b -> p (a b)"))
        eye_sb = consts.tile([128, 128], F32, name="eye_sb")
        nc.gpsimd.dma_start(eye_sb, eye_t.ap())
        eye8 = consts.tile([128, 128], FP8, name="eye8")
        nc.scalar.sign(eye8, eye_sb)

        # ---- first two images' x so conv can start early (SP ring) ----
        xst = []
        for n in range(B_SH):
            xst.append(stage.tile([128, 2, H * H], F32, name=f"xst{n}", tag="xst"))
        for n in range(2):
            nc.sync.dma_start(
                xst[n], x_t.ap()[n].rearrange("(cc p) h w -> p cc (h w)", p=128)
            )

        # ---- weight inputs: fully contiguous [o, c*9+t] loads (SP ring) ----
        z_sb = []
        for k in range(K):
            z_k = zpool.tile([128, CT], F32, name=f"z{k}", tag="z", bufs=4)
            nc.sync.dma_start(z_k, z_t.ap()[k].rearrange("o c ky kx -> o (c ky kx)"))
            z_sb.append(z_k)
        m_sb = zpool.tile([128, CT], F32, name="m_sb")
        nc.sync.dma_start(m_sb, m_t.ap().rearrange("o c ky kx -> o (c ky kx)"))

        # remaining x loads stream behind the weight inputs (ACT ring)
        for n in range(2, B_SH):
            nc.scalar.dma_start(
                xst[n], x_t.ap()[n].rearrange("(cc p) h w -> p cc (h w)", p=128)
            )

        # ---- wsum = M + sum_k rv[k]*Z[k]; muls on ACT, add chain on DVE
        # (sequential k order to match the reference dot's rounding) ----
        acc = wpool.tile([128, CT], F32, name="acc")
        zs_prev = None
        for k in range(K):
            zs_k = wpool.tile([128, CT], F32, name=f"zs{k}", tag="zs", bufs=3)
            nc.scalar.mul(zs_k, z_sb[k], mul=rv_sb[:, k : k + 1])
            if k == 1:
                nc.vector.tensor_add(acc, zs_prev, zs_k)
            elif k > 1:
                nc.vector.tensor_add(acc, acc, zs_k)
            zs_prev = zs_k
        nc.vector.tensor_add(acc, acc, m_sb)

        # ---- binarize + transpose to conv layout ----
        w8 = wpool.tile([128, CT], FP8, name="w8")
        nc.scalar.sign(w8, acc)
        wt = consts.tile([128, KK, 2, 128], FP8, name="wt")
        for t in range(KK):
            for cc in range(2):
                blk = bass.AP(
                    tensor=w8.tensor,
                    offset=w8.offset + cc * 128 * KK + t,
                    ap=[w8.ap[0], [KK, 128]],
                )
                ps_t = pst.tile([128, 128], F32, name="ps_t", tag="ps_t")
                nc.tensor.matmul(ps_t, blk, eye8, start=True, stop=True)
                nc.vector.tensor_copy(wt[:, t, cc, :], ps_t)

        # ---- activations: sign(x) into zero-padded fp8 images ----
        act_tiles = []
        for n in range(B_SH):
            a_n = acts.tile([128, 2, PADW], FP8, name=f"a{n}", tag=f"a{n}")
            # zero only the padding border (top row / bottom row+tail /
            # the 30-column pairs straddling row ends)
            nc.vector.memset(a_n[:, :, 0:30], 0.0)
            nc.vector.memset(a_n[:, :, 870:PADW], 0.0)
            pairs = a_n[:, :, 29 : 29 + 29 * HP].rearrange(
                "p cc (r two) -> p cc r two", two=HP
            )[:, :, :, :2]
            nc.vector.memset(pairs, 0.0)
            interior = a_n[:, :, 31 : 31 + 28 * HP].rearrange(
                "p cc (r xx) -> p cc r xx", xx=HP
            )[:, :, :, :28]
            nc.scalar.sign(interior, xst[n].rearrange("p cc (h w) -> p cc h w", w=28))
            act_tiles.append(a_n)

        # ---- conv: 9 shifted DoubleRow matmuls per (image, half) ----
        group = 0
        for n in range(B_SH):
            a_n = act_tiles[n]
            for half in range(2):
                ps = psums.tile([128, 420], F32, name="ps", tag="ps")
                for t in range(KK):
                    dy, dx = divmod(t, 3)
                    off = (half * 14 + dy) * HP + dx
                    nc.tensor.matmul(
                        ps,
                        wt[:, t],
                        a_n[:, :, off : off + 420],
                        start=(t == 0),
                        stop=(t == KK - 1),
                        perf_mode=mybir.MatmulPerfMode.DoubleRow,
                    )
                ob = outp.tile([128, 392], F32, name="ob", tag="ob")
                ps_v = ps.rearrange("p (r xx) -> p r xx", xx=HP)[:, :, :28]
                ob_v = ob.rearrange("p (r xx) -> p r xx", xx=28)
                # alternate the psum-drain engine to balance ACT/DVE
                if group % 2 == 0:
                    nc.scalar.activation(
                        ob_v,
                        ps_v,
                        mybir.ActivationFunctionType.Copy,
                        scale=alpha_sb[:, 0:1],
                    )
                else:
                    nc.vector.tensor_scalar_mul(ob_v, ps_v, alpha_sb[:, 0:1])
                dst = out_t.ap()[n].rearrange("o h w -> o (h w)")[
                    :, half * 392 : (half + 1) * 392
                ]
                nc.sync.dma_start(dst, ob)
                group += 1


_PROGRAM = None


def build_program():
    global _PROGRAM
    if _PROGRAM is not None:
        return _PROGRAM
    nc = bacc.Bacc(
        "TRN2",
        target_bir_lowering=False,
        debug=False,
        enable_asserts=True,
        num_devices=N_CORES,
    )
    x_t = nc.dram_tensor("x", [B_SH, C, H, H], F32, kind="ExternalInput")
    m_t = nc.dram_tensor("M", [O_SH, C, 3, 3], F32, kind="ExternalInput")
    z_t = nc.dram_tensor("Z", [K, O_SH, C, 3, 3], F32, kind="ExternalInput")
    a_t = nc.dram_tensor("alpha", [O_SH, 1, 1], F32, kind="ExternalInput")
    rv_t = nc.dram_tensor("rv", [1, K], F32, kind="ExternalInput")
    eye_t = nc.inline_tensor(np.eye(128, dtype=np.float32), name="eye128")
    out_t = nc.dram_tensor("out", [B_SH, O_SH, H, H], F32, kind="ExternalOutput")

    with tile.TileContext(nc) as tc:
        _build_kernel(tc, x_t, m_t, z_t, a_t, rv_t, eye_t, out_t)
    nc.compile()
    _PROGRAM = nc
    return nc


def make_in_maps(x, M, Z, alpha, rv):
    x = np.ascontiguousarray(np.asarray(x, dtype=np.float32))
    M = np.ascontiguousarray(np.asarray(M, dtype=np.float32))
    Z = np.ascontiguousarray(np.asarray(Z, dtype=np.float32))
    alpha = np.ascontiguousarray(np.asarray(alpha, dtype=np.float32))
    rv = np.ascontiguousarray(np.asarray(rv, dtype=np.float32))
    in_maps = []
    for i in range(N_CORES):
        b, oh = divmod(i, 2)
        in_maps.append(
            {
                "x": np.ascontiguousarray(x[b * B_SH : (b + 1) * B_SH]),
                "M": np.ascontiguousarray(M[oh * O_SH : (oh + 1) * O_SH]),
                "Z": np.ascontiguousarray(Z[:, oh * O_SH : (oh + 1) * O_SH]),
                "alpha": np.ascontiguousarray(alpha[oh * O_SH : (oh + 1) * O_SH]),
                "rv": rv,
            }
        )
    return in_maps


def assemble_out(results):
    out = np.empty((B, O, H, H), dtype=np.float32)
    for i in range(N_CORES):
        b, oh = divmod(i, 2)
        r = np.asarray(results[i]["out"]).reshape(B_SH, O_SH, H, H)
        out[b * B_SH : (b + 1) * B_SH, oh * O_SH : (oh + 1) * O_SH] = r
    return out


def kernel(x, M, Z, alpha, rv, trace=False):
    nc = build_program()
    in_maps = make_in_maps(x, M, Z, alpha, rv)
    res = run_bass_kernel_spmd(
        nc, in_maps, core_ids=list(range(N_CORES)), trace=trace
    )
    if trace:
        kernel.last_results = res
    return assemble_out(res.results)


if __name__ == "__main__":
    build_program()
    print("program built ok")


# revision 8
# speedup vs baseline: 1.8117x; 1.1366x over previous
"""Trainium2 Bass kernel for BinarizeConv2dSDP.

Reference math (forward only):
    w    = rsqrt(m^2 + sum_k z_k^2/100) * (m + rv @ z)   elementwise
    bw   = sign(w)        -- the positive rsqrt factor drops out of sign()
    ba   = sign(x)
    out  = conv2d(ba, bw, pad=1, NCHW/OIHW) * alpha[o]

Device computation: bw = sign(M + sum_k rv[k]*Z[k]), ba = sign(x), then the
3x3 pad-1 conv as 9 shifted fp8 DoubleRow matmuls accumulating in PSUM
(everything is +-1, so fp8 e4m3 with f32 PSUM accumulation is bit-exact),
alpha folded into the PSUM->SBUF copy.

Sharding (8 cores, no collectives): 2D grid, batch 4-way x out-channel
2-way. Core i handles images [16*(i//2), 16*(i//2)+16) and out-channels
[128*(i%2), 128*(i%2)+128). Each core reads only its Z/M/alpha o-half and
its x batch-quarter; outputs are disjoint.

Per-core layouts:
  - z_k, m, wsum: [128 part(o), 2304 (c*9+t)] f32 -- natural Z order, so
    all weight DMAs are fully contiguous.
  - weight sum on ACT (muls by rv[k]) + DVE (sequential add chain, same
    f32 order as the reference dot).
  - sign -> w8 [128(o), 2304] fp8; 18 PE transposes (matmul with fp8
    identity rhs, lhsT = stride-9 column slice) -> W [128 part(c_low),
    9 tap, 2 c-chunk, 128 o] fp8.
  - activations: per image [128 part(c_low), 2 c-chunk, 912] fp8 zero-
    padded 30x30 images (912 = 900 rounded up to %16 for the DoubleRow
    AP-step constraint); conv output on a 30-wide grid, junk columns
    skipped at the output DMA.
"""

import sys

for _p in ("/opt/trn_rl_repo",):
    if _p not in sys.path:
        sys.path.insert(0, _p)

import contextlib

import numpy as np

import concourse.bass as bass
import concourse.bacc as bacc
import concourse.tile as tile
from concourse import mybir
from concourse.bass_utils import run_bass_kernel_spmd

N_CORES = 8
B = 64
B_SH = 16       # images per core (batch/4)
C = 256         # in channels
O = 256
O_SH = 128      # out channels per core (o/2)
K = 8           # SDP rank
KK = 9          # 3x3 taps
CT = C * KK     # 2304
H = 28
HP = 30         # padded row width
PADW = 912      # 30*30=900 padded to %16
F32 = mybir.dt.float32
FP8 = mybir.dt.float8e4


def _build_kernel(tc, x_t, m_t, z_t, a_t, rv_t, eye_t, out_t):
    nc = tc.nc
    ctx = contextlib.ExitStack()
    consts = ctx.enter_context(tc.tile_pool(name="consts", bufs=1))
    zpool = ctx.enter_context(tc.tile_pool(name="zpool", bufs=1))
    wpool = ctx.enter_context(tc.tile_pool(name="wpool", bufs=1))
    stage = ctx.enter_context(tc.tile_pool(name="stage", bufs=4))
    acts = ctx.enter_context(tc.tile_pool(name="acts", bufs=1))
    outp = ctx.enter_context(tc.tile_pool(name="outp", bufs=4))
    psums = ctx.enter_context(tc.tile_pool(name="psums", bufs=6, space="PSUM"))
    pst = ctx.enter_context(tc.tile_pool(name="pst", bufs=2, space="PSUM"))

    with ctx:
        # ---- tiny constants (gpsimd SWDGE queue; keep HW rings free) ----
        rv_sb = consts.tile([128, K], F32, name="rv_sb")
        nc.gpsimd.dma_start(rv_sb, rv_t.ap().to_broadcast((128, K)))
        alpha_sb = consts.tile([128, 1], F32, name="alpha_sb")
        nc.gpsimd.dma_start(alpha_sb, a_t.ap().rearrange("p a b -> p (a b)"))
        eye_sb = consts.tile([128, 128], F32, name="eye_sb")
        nc.gpsimd.dma_start(eye_sb, eye_t.ap())
        eye8 = consts.tile([128, 128], FP8, name="eye8")
        nc.scalar.sign(eye8, eye_sb)

        # ---- weight inputs first: fully contiguous [o, c*9+t] loads.
        # Everything big goes on the single SP HWDGE ring, in priority
        # order (z+M gate the conv start; x streams behind them). ----
        z_sb = []
        for k in range(K):
            z_k = zpool.tile([128, CT], F32, name=f"z{k}", tag="z", bufs=6)
            nc.sync.dma_start(z_k, z_t.ap()[k].rearrange("o c ky kx -> o (c ky kx)"))
            z_sb.append(z_k)
        m_sb = zpool.tile([128, CT], F32, name="m_sb")
        nc.sync.dma_start(m_sb, m_t.ap().rearrange("o c ky kx -> o (c ky kx)"))

        xst = []
        for n in range(B_SH):
            xst.append(stage.tile([128, 2, H * H], F32, name=f"xst{n}", tag="xst"))
        for n in range(B_SH):
            nc.sync.dma_start(
                xst[n], x_t.ap()[n].rearrange("(cc p) h w -> p cc (h w)", p=128)
            )

        # ---- wsum = M + sum_k rv[k]*Z[k]; muls on ACT, add chain on DVE
        # (sequential k order to match the reference dot's rounding) ----
        acc = wpool.tile([128, CT], F32, name="acc")
        zs_prev = None
        for k in range(K):
            zs_k = wpool.tile([128, CT], F32, name=f"zs{k}", tag="zs", bufs=3)
            nc.scalar.mul(zs_k, z_sb[k], mul=rv_sb[:, k : k + 1])
            if k == 1:
                nc.vector.tensor_add(acc, zs_prev, zs_k)
            elif k > 1:
                nc.vector.tensor_add(acc, acc, zs_k)
            zs_prev = zs_k
        nc.vector.tensor_add(acc, acc, m_sb)

        # ---- binarize + transpose to conv layout ----
        w8 = wpool.tile([128, CT], FP8, name="w8")
        nc.scalar.sign(w8, acc)
        wt = consts.tile([128, KK, 2, 128], FP8, name="wt")
        for t in range(KK):
            for cc in range(2):
                blk = bass.AP(
                    tensor=w8.tensor,
                    offset=w8.offset + cc * 128 * KK + t,
                    ap=[w8.ap[0], [KK, 128]],
                )
                ps_t = pst.tile([128, 128], F32, name="ps_t", tag="ps_t")
                nc.tensor.matmul(ps_t, blk, eye8, start=True, stop=True)
                nc.vector.tensor_copy(wt[:, t, cc, :], ps_t)

        # ---- activations: sign(x) into zero-padded fp8 images ----
        act_tiles = []
        for n in range(B_SH):
            a_n = acts.tile([128, 2, PADW], FP8, name=f"a{n}", tag=f"a{n}")
            # zero only the padding border (top row / bottom row+tail /
            # the 30-column pairs straddling row ends)
            nc.vector.memset(a_n[:, :, 0:30], 0.0)
            nc.vector.memset(a_n[:, :, 870:PADW], 0.0)
            pairs = a_n[:, :, 29 : 29 + 29 * HP].rearrange(
                "p cc (r two) -> p cc r two", two=HP
            )[:, :, :, :2]
            nc.vector.memset(pairs, 0.0)
            interior = a_n[:, :, 31 : 31 + 28 * HP].rearrange(
                "p cc (r xx) -> p cc r xx", xx=HP
            )[:, :, :, :28]
            nc.scalar.sign(interior, xst[n].rearrange("p cc (h w) -> p cc h w", w=28))
            act_tiles.append(a_n)

        # ---- conv: 9 shifted DoubleRow matmuls per (image, half) ----
        group = 0
        for n in range(B_SH):
            a_n = act_tiles[n]
            for half in range(2):
                ps = psums.tile([128, 420], F32, name="ps", tag="ps")
                for t in range(KK):
                    dy, dx = divmod(t, 3)
                    off = (half * 14 + dy) * HP + dx
                    nc.tensor.matmul(
                        ps,
                        wt[:, t],
                        a_n[:, :, off : off + 420],
                        start=(t == 0),
                        stop=(t == KK - 1),
                        perf_mode=mybir.MatmulPerfMode.DoubleRow,
                    )
                ob = outp.tile([128, 392], F32, name="ob", tag="ob")
                ps_v = ps.rearrange("p (r xx) -> p r xx", xx=HP)[:, :, :28]
                ob_v = ob.rearrange("p (r xx) -> p r xx", xx=28)
                # alternate the psum-drain engine to balance ACT/DVE
                if group % 2 == 0:
                    nc.scalar.activation(
                        ob_v,
                        ps_v,
                        mybir.ActivationFunctionType.Copy,
                        scale=alpha_sb[:, 0:1],
                    )
                else:
                    nc.vector.tensor_scalar_mul(ob_v, ps_v, alpha_sb[:, 0:1])
                dst = out_t.ap()[n].rearrange("o h w -> o (h w)")[
                    :, half * 392 : (half + 1) * 392
                ]
                nc.sync.dma_start(dst, ob)
                group += 1


_PROGRAM = None


def build_program():
    global _PROGRAM
    if _PROGRAM is not None:
        return _PROGRAM
    nc = bacc.Bacc(
        "TRN2",
        target_bir_lowering=False,
        debug=False,
        enable_asserts=True,
        num_devices=N_CORES,
    )
    x_t = nc.dram_tensor("x", [B_SH, C, H, H], F32, kind="ExternalInput")
    m_t = nc.dram_tensor("M", [O_SH, C, 3, 3], F32, kind="ExternalInput")
    z_t = nc.dram_tensor("Z", [K, O_SH, C, 3, 3], F32, kind="ExternalInput")
    a_t = nc.dram_tensor("alpha", [O_SH, 1, 1], F32, kind="ExternalInput")
    rv_t = nc.dram_tensor("rv", [1, K], F32, kind="ExternalInput")
    eye_t = nc.inline_tensor(np.eye(128, dtype=np.float32), name="eye128")
    out_t = nc.dram_tensor("out", [B_SH, O_SH, H, H], F32, kind="ExternalOutput")

    with tile.TileContext(nc) as tc:
        _build_kernel(tc, x_t, m_t, z_t, a_t, rv_t, eye_t, out_t)
    nc.compile()
    _PROGRAM = nc
    return nc


def make_in_maps(x, M, Z, alpha, rv):
    x = np.ascontiguousarray(np.asarray(x, dtype=np.float32))
    M = np.ascontiguousarray(np.asarray(M, dtype=np.float32))
    Z = np.ascontiguousarray(np.asarray(Z, dtype=np.float32))
    alpha = np.ascontiguousarray(np.asarray(alpha, dtype=np.float32))
    rv = np.ascontiguousarray(np.asarray(rv, dtype=np.float32))
    in_maps = []
    for i in range(N_CORES):
        b, oh = divmod(i, 2)
        in_maps.append(
            {
                "x": np.ascontiguousarray(x[b * B_SH : (b + 1) * B_SH]),
                "M": np.ascontiguousarray(M[oh * O_SH : (oh + 1) * O_SH]),
                "Z": np.ascontiguousarray(Z[:, oh * O_SH : (oh + 1) * O_SH]),
                "alpha": np.ascontiguousarray(alpha[oh * O_SH : (oh + 1) * O_SH]),
                "rv": rv,
            }
        )
    return in_maps


def assemble_out(results):
    out = np.empty((B, O, H, H), dtype=np.float32)
    for i in range(N_CORES):
        b, oh = divmod(i, 2)
        r = np.asarray(results[i]["out"]).reshape(B_SH, O_SH, H, H)
        out[b * B_SH : (b + 1) * B_SH, oh * O_SH : (oh + 1) * O_SH] = r
    return out


def kernel(x, M, Z, alpha, rv, trace=False):
    nc = build_program()
    in_maps = make_in_maps(x, M, Z, alpha, rv)
    res = run_bass_kernel_spmd(
        nc, in_maps, core_ids=list(range(N_CORES)), trace=trace
    )
    if trace:
        kernel.last_results = res
    return assemble_out(res.results)


if __name__ == "__main__":
    build_program()
    print("program built ok")
